# revision 16
# baseline (speedup 1.0000x reference)
"""MetaOptNet SVM-CS head on 8 Trainium2 NeuronCores.

Math: the reference runs a 15-iteration Mehrotra interior-point solve of the
Crammer-Singer dual QP per task. Empirically (f64 replication) the IPM is
fully converged by iteration 15, so the target equals the QP optimum. We
compute that optimum with a fixed-matrix ADMM:

    per task:  K = S S^T  (25x25 Gram)
               W~ = rho * (K + (1+rho) I)^{-1}   (Newton-Schulz, 3 iters:
                   2 in bf16 + 1 fp32 polish; |I - cH| <= ~0.1 since
                   9 <= eig(K+9I) <= ~17, and the final fp32 iteration
                   squares the bf16 error away)
               10x ADMM (rho=8), in (d1 = u-y, oy = y+oh/rho) state form:
                   t = center_ways(W~ @ d1) + oy
                   d1' = min(t, 2h - t);  oy' = max(t - (h - oh/rho), oh/rho)
                   where h = (C + 1/rho) oh
               logits = compat^T @ x * scale    (x = center_ways(W~ @ d1),
                   compat = S Q^T computed as one batched PE Gram per group)

The equality constraint A z = 0 (sum over ways per sample) reduces to
centering across ways because A A^T = n_way I; the KKT matrix is way-block-
diagonal with identical blocks K + (1+rho)I, which is what makes the single
25x25 inverse per task sufficient.

Instruction-count shape (the v1 kernel was PE-instruction-overhead bound at
~105us HW: 1072 matmuls of ~200ns each): tasks sit in 32-aligned 25-row
blocks, four tasks per 128-partition tile. Per group the support Gram and
the query/support compat are SINGLE 20-chunk accumulation chains over the
full [128,128] / [128,300] tiles (cross-task blocks are zeroed by a DVE
mask for the Gram, and simply never read for compat), instead of per-task
tile_position matmuls. logits come from compat^T x (contract over the 25
support samples) instead of Q (S^T x) (contract over d=2560), which deletes
the 640 five-column matmuls of v1. Total matmul count ~240 instead of 1072.

Sharding: pure data parallel, 16 tasks per core. Host-side work is layout
only (shard, transpose packing into 128-partition DMA tiles, one-hot
constants); all FLOPs run on-device.

Precision: the QP (Gram, inverse, ADMM) runs in fp32. S/Q ship and contract
in bf16, compat is carried in bf16 — the error enters the output only
linearly (~4e-3 relative on the logits, tolerance 2e-2).

DMA: st/qt are single contiguous [128, n] bf16 tensors (20KB / 48KB per
partition line -> large hardware-DGE descriptors, no small-packet software
descriptor storm); all f32 constants ride in one [128, 316] tensor
(mask | blockdiag-eye | ohc | h2 | hmo); the scalar multiples of blockdiag-I
(2I for Newton-Schulz, 9I for H, 0.065I for X0) are derived on-device.
"""

import sys

sys.path.insert(0, "/opt/trn_rl_repo")

from contextlib import ExitStack

import numpy as np

import concourse.bass as bass
import concourse.tile as tile
from concourse import mybir
from concourse.alu_op_type import AluOpType
from concourse.bass_utils import run_bass_kernel_spmd
from concourse.tile import TileContext

# ---------------------------------------------------------------------------
# Problem constants (hardcoded per the harness contract)
N_CORES = 8
B_TOT = 128
T = 16            # tasks per core
NS = 25           # support samples per task
NW = 5            # ways
NQ = 75           # queries per task
D = 2560          # feature dim
NCH = D // 128    # 20 d-chunks
G = 4             # task groups per core (4 tasks each -> 128-partition tiles)
GP = T // G       # tasks per group
GC = GP * 32      # 128 block columns per group in st (25 used per task)
GQ = GP * NQ      # 300 query columns per group
RHO = 8.0
NS_C = 0.065      # Newton-Schulz init scale for H = K + 9I
NS_ITERS = 3
ADMM_ITERS = 10
C_REG = 0.1

F32 = mybir.dt.float32
BF16 = mybir.dt.bfloat16

# consts_f32 [128, 316] column map
C_MASK = 0        # [128,128] block-diag 25x25 ones at 32-offsets
C_EYE = 128       # [128,128] block-diag I_25 at 32-offsets
C_OHC = 256      # [128,20] oh / rho
C_H2 = 276       # [128,20] 2*(C+1/rho)*oh
C_HMO = 296      # [128,20] C*oh


# ---------------------------------------------------------------------------
# The walrus build here encodes at most ONE sync-wait command per instruction
# (TPB_CTRL / S3_LW setupSyncWait raises "Too many sync wait commands").
# Tile's scheduler freely attaches several waits to one instruction, so after
# scheduling we split the excess onto NoOps inserted immediately before the
# instruction on the same engine — identical semantics, encodable waits.
def _split_waits(nc, max_waits=1):
    cnt = 0
    for blk in nc.m.functions[0].blocks:
        insns = blk.instructions
        idx = 0
        while idx < len(insns):
            ins = insns[idx]
            si = ins.sync_info
            waits = list(si.on_wait) if si and si.on_wait else []
            if len(waits) > max_waits:
                si.on_wait = waits[:max_waits]
                for w in waits[max_waits:]:
                    nop = mybir.InstNoOp(name=f"waitnop_{cnt}", ins=[], outs=[])
                    cnt += 1
                    nop.engine = ins.engine
                    nop.sync_info = mybir.SyncInfo(on_wait=[w], on_update=[])
                    nc.register_instruction(nop, overwrite=True)
                    insns.insert(idx, nop)
                    idx += 1
            idx += 1
    return cnt


# ---------------------------------------------------------------------------
def _build_program():
    nc = bass.Bass("TRN2", target_bir_lowering=False)

    st_d = nc.dram_tensor("st", [128, NCH * G * GC], BF16, kind="ExternalInput")
    qt_d = nc.dram_tensor("qt", [128, NCH * T * NQ], BF16, kind="ExternalInput")
    cf_d = nc.dram_tensor("cf", [128, 316], F32, kind="ExternalInput")
    scale_d = nc.dram_tensor("scale", [1, 1], F32, kind="ExternalInput")
    out_d = nc.dram_tensor("out", [NQ, T * NW], F32, kind="ExternalOutput")

    def stsl(tile_, c, g):
        base = g * (NCH * GC) + c * GC
        return tile_[:, base : base + GC]

    def qtsl(tile_, c, g):
        base = g * (NCH * GQ) + c * GQ
        return tile_[:, base : base + GQ]

    with ExitStack() as ctx:
        tc = ctx.enter_context(TileContext(nc))
        iw_pool = ctx.enter_context(tc.tile_pool(name="iw", bufs=1))
        consts = ctx.enter_context(tc.tile_pool(name="consts", bufs=1))
        mats = ctx.enter_context(tc.tile_pool(name="mats", bufs=12))
        state = ctx.enter_context(tc.tile_pool(name="state", bufs=10))
        cmp_pool = ctx.enter_context(tc.tile_pool(name="cmp", bufs=G))

        # ---- loads --------------------------------------------------------
        cf_sb = consts.tile([128, 316], F32, tag="cf")
        nc.sync.dma_start(out=cf_sb, in_=cf_d[:, :])
        mask_sb = cf_sb[:, C_MASK : C_MASK + 128]
        beye_sb = cf_sb[:, C_EYE : C_EYE + 128]
        ohc_sb = cf_sb[:, C_OHC : C_OHC + 20]
        h2_sb = cf_sb[:, C_H2 : C_H2 + 20]
        hmo_sb = cf_sb[:, C_HMO : C_HMO + 20]

        # chunked DMAs across both trigger queues: moderate per-partition
        # lines (2.5-4KB) keep many partitions streaming concurrently and
        # many independent transfers keep all 16 DMA queues fed (one big
        # transfer measured only ~90-110GB/s; v1-style chunking ~270GB/s).
        # g-major layout so each group's compute can start as its slice lands.
        st_tile = iw_pool.tile([128, NCH * G * GC], BF16, tag="st")
        qt_tile = iw_pool.tile([128, NCH * T * NQ], BF16, tag="qt")
        eng = [nc.sync, nc.gpsimd]
        ei = 0
        stw = NCH * GC          # 2560 cols per group; halves of 2.5KB/part
        for g in range(G):
            for hf in range(2):
                sl_ = slice(g * stw + hf * (stw // 2), g * stw + (hf + 1) * (stw // 2))
                eng[ei % 2].dma_start(out=st_tile[:, sl_], in_=st_d[:, sl_])
                ei += 1
        qtw = NCH * GQ          # 6000 cols per group; 10 chunks of 1.2KB/part
        for g in range(G):
            for th in range(10):
                sl_ = slice(g * qtw + th * (qtw // 10), g * qtw + (th + 1) * (qtw // 10))
                eng[ei % 2].dma_start(out=qt_tile[:, sl_], in_=qt_d[:, sl_])
                ei += 1

        scale_sb = consts.tile([NQ, 1], F32, tag="scale")
        nc.gpsimd.dma_start(out=scale_sb, in_=scale_d[:, :].to_broadcast([NQ, 1]))

        # bf16 Newton-Schulz init X0 = NS_C * blockdiag-I, derived from beye
        cib_sb = consts.tile([128, 128], BF16, tag="cib")
        nc.scalar.activation(
            cib_sb, beye_sb, mybir.ActivationFunctionType.Copy, scale=NS_C
        )

        # ADMM state: d1 = u - y (init ohc), oy = y + ohc (init ohc)
        d1_sb = state.tile([128, 20], F32, tag="d1")
        nc.vector.tensor_copy(d1_sb, ohc_sb)
        oy_sb = state.tile([128, 20], F32, tag="oy")
        nc.vector.tensor_copy(oy_sb, ohc_sb)

        # ---- stage 1: H_g = mask .* (S S^T) + 9*beye, one chain per group -
        h_all = []
        hb_all = []
        with tc.tile_pool(name="kpsum", bufs=4, space="PSUM") as kpsum:
            for g in range(G):
                kp = kpsum.tile([128, 128], F32, tag="kp")
                for c in range(NCH):
                    nc.tensor.matmul(
                        kp,
                        lhsT=stsl(st_tile, c, g),
                        rhs=stsl(st_tile, c, g),
                        start=(c == 0),
                        stop=(c == NCH - 1),
                    )
                km = mats.tile([128, 128], F32, tag="km")
                nc.vector.tensor_tensor(km, kp, mask_sb, op=AluOpType.mult)
                h_sb = mats.tile([128, 128], F32, tag="h")
                nc.vector.scalar_tensor_tensor(
                    out=h_sb,
                    in0=beye_sb,
                    scalar=1.0 + RHO,
                    in1=km,
                    op0=AluOpType.mult,
                    op1=AluOpType.add,
                )
                h_all.append(h_sb)
                hb = mats.tile([128, 128], BF16, tag="hb")
                nc.vector.tensor_copy(hb, h_sb)
                hb_all.append(hb)

        # ---- stage 2: Newton-Schulz inverse, 4 groups pipelined -----------
        # iters 0-1 in bf16 (NS self-corrects), final iter fp32 squares the
        # bf16 error away (~0.4%^2), so W~ is fp32-quality at 1/4 PE cost.
        wt_sb = []
        with tc.tile_pool(name="npsum", bufs=4, space="PSUM") as npsum:
            x_cur = [cib_sb] * G
            for it in range(NS_ITERS):
                last = it == NS_ITERS - 1
                prev_last = it == NS_ITERS - 2
                for g in range(G):
                    t1p = npsum.tile([128, 128], F32, tag="t1p")
                    nc.tensor.matmul(
                        t1p,
                        lhsT=h_all[g] if last else hb_all[g],
                        rhs=x_cur[g],
                        start=True,
                        stop=True,
                    )
                    u_ns = mats.tile(
                        [128, 128], F32 if last else BF16,
                        tag="u_ns" if last else "u_nsb",
                    )
                    # u = 2*beye - H X
                    nc.vector.scalar_tensor_tensor(
                        out=u_ns,
                        in0=beye_sb,
                        scalar=2.0,
                        in1=t1p,
                        op0=AluOpType.mult,
                        op1=AluOpType.subtract,
                    )
                    x2p = npsum.tile([128, 128], F32, tag="x2p")
                    nc.tensor.matmul(
                        x2p, lhsT=x_cur[g], rhs=u_ns, start=True, stop=True
                    )
                    if last:
                        wt = mats.tile([128, 128], F32, tag="wt")
                        nc.scalar.activation(
                            wt, x2p, mybir.ActivationFunctionType.Copy, scale=RHO
                        )
                        wt_sb.append(wt)
                    else:
                        x_next = mats.tile(
                            [128, 128], F32 if prev_last else BF16,
                            tag="x_ns" if prev_last else "x_nsb",
                        )
                        nc.scalar.activation(
                            x_next, x2p, mybir.ActivationFunctionType.Copy
                        )
                        x_cur[g] = x_next

        # ---- stage 3: ADMM (d1/oy state form) -----------------------------
        # t = center(Wt @ d1) + y + OHC;  d1' = min(t, 2h-t);
        # oy' = max(t - (h - OHC), OHC)
        xb_sb = None
        mpsum = ctx.enter_context(tc.tile_pool(name="mpsum", bufs=2, space="PSUM"))
        lpsum = ctx.enter_context(tc.tile_pool(name="lpsum", bufs=3, space="PSUM"))
        for it in range(ADMM_ITERS):
            xp = mpsum.tile([128, 20], F32, tag="mp")
            for g in range(G):
                nc.tensor.matmul(
                    xp[:, g * NW : (g + 1) * NW],
                    lhsT=wt_sb[g],
                    rhs=d1_sb[:, g * NW : (g + 1) * NW],
                    start=True,
                    stop=True,
                )
            msum = state.tile([128, 4], F32, tag="msum")
            nc.vector.reduce_sum(
                msum,
                xp[:, :].rearrange("p (g w) -> p g w", w=NW),
                axis=mybir.AxisListType.X,
            )
            msb = msum[:, :]
            msb_ap = bass.AP(
                tensor=msb.tensor, offset=msb.offset, ap=[msb.ap[0], msb.ap[1], [0, NW]]
            )
            p1 = state.tile([128, 20], F32, tag="p1")
            nc.vector.tensor_tensor(p1, xp, oy_sb, op=AluOpType.add)
            tt_sb = state.tile([128, 20], F32, tag="tt")
            nc.vector.scalar_tensor_tensor(
                out=tt_sb[:, :].rearrange("p (g w) -> p g w", w=NW),
                in0=msb_ap,
                scalar=-1.0 / NW,
                in1=p1[:, :].rearrange("p (g w) -> p g w", w=NW),
                op0=AluOpType.mult,
                op1=AluOpType.add,
            )
            if it == ADMM_ITERS - 1:
                xb_sb = state.tile([128, 20], BF16, tag="xb")
                nc.vector.scalar_tensor_tensor(
                    out=xb_sb[:, :].rearrange("p (g w) -> p g w", w=NW),
                    in0=msb_ap,
                    scalar=-1.0 / NW,
                    in1=xp[:, :].rearrange("p (g w) -> p g w", w=NW),
                    op0=AluOpType.mult,
                    op1=AluOpType.add,
                )
            n2h = state.tile([128, 20], F32, tag="n2h")
            nc.vector.scalar_tensor_tensor(
                out=n2h,
                in0=tt_sb,
                scalar=-1.0,
                in1=h2_sb,
                op0=AluOpType.mult,
                op1=AluOpType.add,
            )
            d1_sb = state.tile([128, 20], F32, tag="d1n")
            nc.vector.tensor_tensor(d1_sb, tt_sb, n2h, op=AluOpType.min)
            if it < ADMM_ITERS - 1:
                pa = state.tile([128, 20], F32, tag="pa")
                nc.vector.tensor_tensor(pa, tt_sb, hmo_sb, op=AluOpType.subtract)
                oy_sb = state.tile([128, 20], F32, tag="oy2")
                nc.vector.tensor_tensor(oy_sb, pa, ohc_sb, op=AluOpType.max)

        # ---- stage 4: compat = S Q^T (after ADMM: overlaps the qt DMA) ----
        # one [128,300] chain per group; rows = group support (25-in-32
        # blocks), cols = group queries (4 x 75). Off-diagonal 25x75 blocks
        # are cross-task garbage, never read downstream.
        compat_sb = []
        with tc.tile_pool(name="cpsum", bufs=2, space="PSUM") as cpsum:
            for g in range(G):
                cp = cpsum.tile([128, GQ], F32, tag="cp")
                for c in range(NCH):
                    nc.tensor.matmul(
                        cp,
                        lhsT=stsl(st_tile, c, g),
                        rhs=qtsl(qt_tile, c, g),
                        start=(c == 0),
                        stop=(c == NCH - 1),
                    )
                cb = cmp_pool.tile([128, GQ], BF16, tag="cb")
                nc.vector.tensor_copy(cb, cp)
                compat_sb.append(cb)

        # ---- stage 5: logits[q, t*5+w] = sum_s compat[s,q] x[s,w] ---------
        out_sb = consts.tile([NQ, T * NW], F32, tag="outsb")
        for t in range(T):
            g, tp = t // GP, t % GP
            rs = slice(tp * 32, tp * 32 + NS)
            qs = slice(tp * NQ, (tp + 1) * NQ)
            lp = lpsum.tile([NQ, NW], F32, tag="lp")
            nc.tensor.matmul(
                lp,
                lhsT=compat_sb[g][rs, qs],
                rhs=xb_sb[rs, g * NW : (g + 1) * NW],
                start=True,
                stop=True,
                tile_position=(tp * 32, 0),
            )
            nc.scalar.activation(
                out_sb[:, t * NW : (t + 1) * NW],
                lp,
                mybir.ActivationFunctionType.Copy,
                scale=scale_sb,
            )
        nc.sync.dma_start(out=out_d[:, :], in_=out_sb)

    _split_waits(nc)
    return nc


_NC_CACHE = None


def _get_nc():
    global _NC_CACHE
    if _NC_CACHE is None:
        _NC_CACHE = _build_program()
    return _NC_CACHE


# ---------------------------------------------------------------------------
def _host_prep(support, query, support_labels, scale):
    """Shard + pack into the DMA layouts. Layout only, no FLOPs."""
    f32 = np.float32
    bf = mybir.dt.np(BF16)
    blk1 = np.zeros((32, 32), dtype=f32)
    blk1[:NS, :NS] = 1.0
    mask = np.kron(np.eye(GP, dtype=f32), blk1)
    blkI = np.zeros((32, 32), dtype=f32)
    blkI[:NS, :NS] = np.eye(NS, dtype=f32)
    beye = np.kron(np.eye(GP, dtype=f32), blkI)
    sc = np.asarray(scale, dtype=f32).reshape(1, 1)

    in_maps = []
    for core in range(N_CORES):
        sl = slice(core * T, (core + 1) * T)
        S = np.asarray(support[sl], dtype=f32)        # [16,25,2560]
        Q = np.asarray(query[sl], dtype=f32)          # [16,75,2560]
        lab = np.asarray(support_labels[sl])          # [16,25] int
        # st col = g*2560 + c*128 + tp*32 + s  (cols 25..31 of each task 0)
        sta = np.zeros((128, G, NCH, GP, 32), dtype=bf)
        sta[..., :NS] = S.reshape(G, GP, NS, NCH, 128).transpose(4, 0, 3, 1, 2)
        st = np.ascontiguousarray(sta.reshape(128, NCH * G * GC))
        # qt col = g*6000 + c*300 + tp*75 + q
        qt = np.ascontiguousarray(
            Q.reshape(G, GP, NQ, NCH, 128).transpose(4, 0, 3, 1, 2)
            .reshape(128, NCH * T * NQ).astype(bf)
        )
        oh = (lab[:, :, None] == np.arange(NW)[None, None, :]).astype(f32)
        # [16,25,5] -> [128,20]: row = tp*32+s, col = g*5+w
        ohp = np.zeros((GP, 32, G, NW), dtype=f32)
        ohp[:, :NS] = oh.reshape(G, GP, NS, NW).transpose(1, 2, 0, 3)
        ohm = ohp.reshape(128, G * NW)
        cf = np.zeros((128, 316), dtype=f32)
        cf[:, C_MASK : C_MASK + 128] = mask
        cf[:, C_EYE : C_EYE + 128] = beye
        cf[:, C_OHC : C_OHC + 20] = ohm / RHO
        cf[:, C_H2 : C_H2 + 20] = 2.0 * (C_REG + 1.0 / RHO) * ohm
        cf[:, C_HMO : C_HMO + 20] = C_REG * ohm
        in_maps.append(
            {
                "st": st,
                "qt": qt,
                "cf": np.ascontiguousarray(cf),
                "scale": sc,
            }
        )
    return in_maps


def _gather(results):
    """Per-core out maps -> full [B_TOT, NQ, NW] logits."""
    outs = []
    for core in range(N_CORES):
        o = np.asarray(results[core]["out"])          # [75, 80]
        outs.append(o.reshape(NQ, T, NW).transpose(1, 0, 2))
    return np.ascontiguousarray(np.concatenate(outs, axis=0), dtype=np.float32)


def kernel(query, support, scale, support_labels, n_way, n_shot):
    assert int(n_way) == NW and int(n_shot) * int(n_way) == NS
    assert query.shape == (B_TOT, NQ, D) and support.shape == (B_TOT, NS, D)
    nc = _get_nc()
    in_maps = _host_prep(support, query, support_labels, scale)
    res = run_bass_kernel_spmd(nc, in_maps, core_ids=list(range(N_CORES)))
    return _gather(res.results)


# revision 17
# speedup vs baseline: 1.1675x; 1.1675x over previous
"""MetaOptNet SVM-CS head on 8 Trainium2 NeuronCores.

Math: the reference runs a 15-iteration Mehrotra interior-point solve of the
Crammer-Singer dual QP per task. Empirically (f64 replication) the IPM is
fully converged by iteration 15, so the target equals the QP optimum. We
compute that optimum with a fixed-matrix ADMM:

    per task:  K = S S^T  (25x25 Gram)
               W~ = rho * (K + (1+rho) I)^{-1}   (Newton-Schulz, 3 iters:
                   2 in bf16 + 1 fp32 polish; |I - cH| <= ~0.1 since
                   9 <= eig(K+9I) <= ~17, and the final fp32 iteration
                   squares the bf16 error away)
               10x ADMM (rho=8), in (d1 = u-y, oy = y+oh/rho) state form:
                   t = center_ways(W~ @ d1) + oy
                   d1' = min(t, 2h - t);  oy' = max(t - (h - oh/rho), oh/rho)
                   where h = (C + 1/rho) oh
               logits = compat^T @ x * scale    (x = center_ways(W~ @ d1),
                   compat = S Q^T computed as one batched PE Gram per group)

The equality constraint A z = 0 (sum over ways per sample) reduces to
centering across ways because A A^T = n_way I; the KKT matrix is way-block-
diagonal with identical blocks K + (1+rho)I, which is what makes the single
25x25 inverse per task sufficient.

Instruction-count shape (the v1 kernel was PE-instruction-overhead bound at
~105us HW: 1072 matmuls of ~200ns each): tasks sit in 32-aligned 25-row
blocks, four tasks per 128-partition tile. Per group the support Gram and
the query/support compat are SINGLE 20-chunk accumulation chains over the
full [128,128] / [128,300] tiles (cross-task blocks are zeroed by a DVE
mask for the Gram, and simply never read for compat), instead of per-task
tile_position matmuls. logits come from compat^T x (contract over the 25
support samples) instead of Q (S^T x) (contract over d=2560), which deletes
the 640 five-column matmuls of v1. Total matmul count ~240 instead of 1072.

Sharding: pure data parallel, 16 tasks per core. Host-side work is layout
only (shard, transpose packing into 128-partition DMA tiles, one-hot
constants); all FLOPs run on-device.

Precision: the QP (Gram, inverse, ADMM) runs in fp32. S/Q ship and contract
in bf16, compat is carried in bf16 — the error enters the output only
linearly (~4e-3 relative on the logits, tolerance 2e-2).

DMA: st/qt are single contiguous [128, n] bf16 tensors (20KB / 48KB per
partition line -> large hardware-DGE descriptors, no small-packet software
descriptor storm); all f32 constants ride in one [128, 316] tensor
(mask | blockdiag-eye | ohc | h2 | hmo); the scalar multiples of blockdiag-I
(2I for Newton-Schulz, 9I for H, 0.065I for X0) are derived on-device.
"""

import sys

sys.path.insert(0, "/opt/trn_rl_repo")

from contextlib import ExitStack

import numpy as np

import concourse.bass as bass
import concourse.tile as tile
from concourse import mybir
from concourse.alu_op_type import AluOpType
from concourse.bass_utils import run_bass_kernel_spmd
from concourse.tile import TileContext

# ---------------------------------------------------------------------------
# Problem constants (hardcoded per the harness contract)
N_CORES = 8
B_TOT = 128
T = 16            # tasks per core
NS = 25           # support samples per task
NW = 5            # ways
NQ = 75           # queries per task
D = 2560          # feature dim
NCH = D // 128    # 20 d-chunks
G = 4             # task groups per core (4 tasks each -> 128-partition tiles)
GP = T // G       # tasks per group
GC = GP * 32      # 128 block columns per group in st (25 used per task)
GQ = GP * NQ      # 300 query columns per group
RHO = 8.0
NS_C = 0.065      # Newton-Schulz init scale for H = K + 9I
NS_ITERS = 3
ADMM_ITERS = 10
C_REG = 0.1

F32 = mybir.dt.float32
BF16 = mybir.dt.bfloat16

# consts_f32 [128, 316] column map
C_MASK = 0        # [128,128] block-diag 25x25 ones at 32-offsets
C_EYE = 128       # [128,128] block-diag I_25 at 32-offsets
C_OHC = 256      # [128,20] oh / rho
C_H2 = 276       # [128,20] 2*(C+1/rho)*oh
C_HMO = 296      # [128,20] C*oh


# ---------------------------------------------------------------------------
# The walrus build here encodes at most ONE sync-wait command per instruction
# (TPB_CTRL / S3_LW setupSyncWait raises "Too many sync wait commands").
# Tile's scheduler freely attaches several waits to one instruction, so after
# scheduling we split the excess onto NoOps inserted immediately before the
# instruction on the same engine — identical semantics, encodable waits.
def _split_waits(nc, max_waits=1):
    cnt = 0
    for blk in nc.m.functions[0].blocks:
        insns = blk.instructions
        idx = 0
        while idx < len(insns):
            ins = insns[idx]
            si = ins.sync_info
            waits = list(si.on_wait) if si and si.on_wait else []
            if len(waits) > max_waits:
                si.on_wait = waits[:max_waits]
                for w in waits[max_waits:]:
                    nop = mybir.InstNoOp(name=f"waitnop_{cnt}", ins=[], outs=[])
                    cnt += 1
                    nop.engine = ins.engine
                    nop.sync_info = mybir.SyncInfo(on_wait=[w], on_update=[])
                    nc.register_instruction(nop, overwrite=True)
                    insns.insert(idx, nop)
                    idx += 1
            idx += 1
    return cnt


# ---------------------------------------------------------------------------
def _build_program():
    nc = bass.Bass("TRN2", target_bir_lowering=False)

    st_d = nc.dram_tensor("st", [128, NCH * G * GC], BF16, kind="ExternalInput")
    qt_d = nc.dram_tensor("qt", [128, NCH * T * NQ], BF16, kind="ExternalInput")
    cf_d = nc.dram_tensor("cf", [128, 316], F32, kind="ExternalInput")
    scale_d = nc.dram_tensor("scale", [1, 1], F32, kind="ExternalInput")
    out_d = nc.dram_tensor("out", [NQ, T * NW], F32, kind="ExternalOutput")

    def stsl(tile_, c, g):
        base = g * (NCH * GC) + c * GC
        return tile_[:, base : base + GC]

    def qtsl(tile_, c, g):
        base = g * (NCH * GQ) + c * GQ
        return tile_[:, base : base + GQ]

    with ExitStack() as ctx:
        tc = ctx.enter_context(TileContext(nc))
        iw_pool = ctx.enter_context(tc.tile_pool(name="iw", bufs=1))
        consts = ctx.enter_context(tc.tile_pool(name="consts", bufs=1))
        mats = ctx.enter_context(tc.tile_pool(name="mats", bufs=12))
        state = ctx.enter_context(tc.tile_pool(name="state", bufs=10))
        cmp_pool = ctx.enter_context(tc.tile_pool(name="cmp", bufs=G))

        # ---- loads --------------------------------------------------------
        cf_sb = consts.tile([128, 316], F32, tag="cf")
        nc.sync.dma_start(out=cf_sb, in_=cf_d[:, :])
        mask_sb = cf_sb[:, C_MASK : C_MASK + 128]
        beye_sb = cf_sb[:, C_EYE : C_EYE + 128]
        ohc_sb = cf_sb[:, C_OHC : C_OHC + 20]
        h2_sb = cf_sb[:, C_H2 : C_H2 + 20]
        hmo_sb = cf_sb[:, C_HMO : C_HMO + 20]

        # chunked DMAs across both trigger queues: moderate per-partition
        # lines (2.5-4KB) keep many partitions streaming concurrently and
        # many independent transfers keep all 16 DMA queues fed (one big
        # transfer measured only ~90-110GB/s; v1-style chunking ~270GB/s).
        # g-major layout so each group's compute can start as its slice lands.
        st_tile = iw_pool.tile([128, NCH * G * GC], BF16, tag="st")
        qt_tile = iw_pool.tile([128, NCH * T * NQ], BF16, tag="qt")
        eng = [nc.sync, nc.gpsimd]
        ei = 0
        stw = NCH * GC          # 2560 cols per group (5KB/partition)
        for g in range(G):
            sl_ = slice(g * stw, (g + 1) * stw)
            eng[ei % 2].dma_start(out=st_tile[:, sl_], in_=st_d[:, sl_])
            ei += 1
        qtw = NCH * GQ          # 6000 cols per group; 5 chunks of 2.4KB/part
        for g in range(G):
            for th in range(5):
                sl_ = slice(g * qtw + th * (qtw // 5), g * qtw + (th + 1) * (qtw // 5))
                eng[ei % 2].dma_start(out=qt_tile[:, sl_], in_=qt_d[:, sl_])
                ei += 1

        scale_sb = consts.tile([NQ, 1], F32, tag="scale")
        nc.gpsimd.dma_start(out=scale_sb, in_=scale_d[:, :].to_broadcast([NQ, 1]))

        # bf16 Newton-Schulz init X0 = NS_C * blockdiag-I, derived from beye
        cib_sb = consts.tile([128, 128], BF16, tag="cib")
        nc.scalar.activation(
            cib_sb, beye_sb, mybir.ActivationFunctionType.Copy, scale=NS_C
        )

        # ADMM state: d1 = u - y (init ohc), oy = y + ohc (init ohc)
        d1_sb = state.tile([128, 20], F32, tag="d1")
        nc.vector.tensor_copy(d1_sb, ohc_sb)
        oy_sb = state.tile([128, 20], F32, tag="oy")
        nc.vector.tensor_copy(oy_sb, ohc_sb)

        # ---- stage 1: H_g = mask .* (S S^T) + 9*beye, one chain per group -
        h_all = []
        hb_all = []
        with tc.tile_pool(name="kpsum", bufs=4, space="PSUM") as kpsum:
            for g in range(G):
                kp = kpsum.tile([128, 128], F32, tag="kp")
                for c in range(NCH):
                    nc.tensor.matmul(
                        kp,
                        lhsT=stsl(st_tile, c, g),
                        rhs=stsl(st_tile, c, g),
                        start=(c == 0),
                        stop=(c == NCH - 1),
                    )
                km = mats.tile([128, 128], F32, tag="km")
                nc.vector.tensor_tensor(km, kp, mask_sb, op=AluOpType.mult)
                h_sb = mats.tile([128, 128], F32, tag="h")
                nc.vector.scalar_tensor_tensor(
                    out=h_sb,
                    in0=beye_sb,
                    scalar=1.0 + RHO,
                    in1=km,
                    op0=AluOpType.mult,
                    op1=AluOpType.add,
                )
                h_all.append(h_sb)
                hb = mats.tile([128, 128], BF16, tag="hb")
                nc.vector.tensor_copy(hb, h_sb)
                hb_all.append(hb)

        # ---- stage 2: Newton-Schulz inverse, 4 groups pipelined -----------
        # iters 0-1 in bf16 (NS self-corrects), final iter fp32 squares the
        # bf16 error away (~0.4%^2), so W~ is fp32-quality at 1/4 PE cost.
        wt_sb = []
        with tc.tile_pool(name="npsum", bufs=4, space="PSUM") as npsum:
            x_cur = [cib_sb] * G
            for it in range(NS_ITERS):
                last = it == NS_ITERS - 1
                prev_last = it == NS_ITERS - 2
                for g in range(G):
                    t1p = npsum.tile([128, 128], F32, tag="t1p")
                    nc.tensor.matmul(
                        t1p,
                        lhsT=h_all[g] if last else hb_all[g],
                        rhs=x_cur[g],
                        start=True,
                        stop=True,
                    )
                    u_ns = mats.tile(
                        [128, 128], F32 if last else BF16,
                        tag="u_ns" if last else "u_nsb",
                    )
                    # u = 2*beye - H X
                    nc.vector.scalar_tensor_tensor(
                        out=u_ns,
                        in0=beye_sb,
                        scalar=2.0,
                        in1=t1p,
                        op0=AluOpType.mult,
                        op1=AluOpType.subtract,
                    )
                    x2p = npsum.tile([128, 128], F32, tag="x2p")
                    nc.tensor.matmul(
                        x2p, lhsT=x_cur[g], rhs=u_ns, start=True, stop=True
                    )
                    if last:
                        wt = mats.tile([128, 128], F32, tag="wt")
                        nc.scalar.activation(
                            wt, x2p, mybir.ActivationFunctionType.Copy, scale=RHO
                        )
                        wt_sb.append(wt)
                    else:
                        x_next = mats.tile(
                            [128, 128], F32 if prev_last else BF16,
                            tag="x_ns" if prev_last else "x_nsb",
                        )
                        nc.scalar.activation(
                            x_next, x2p, mybir.ActivationFunctionType.Copy
                        )
                        x_cur[g] = x_next

        # ---- stage 3: ADMM (d1/oy state form) -----------------------------
        # t = center(Wt @ d1) + y + OHC;  d1' = min(t, 2h-t);
        # oy' = max(t - (h - OHC), OHC)
        xb_sb = None
        mpsum = ctx.enter_context(tc.tile_pool(name="mpsum", bufs=2, space="PSUM"))
        lpsum = ctx.enter_context(tc.tile_pool(name="lpsum", bufs=3, space="PSUM"))
        for it in range(ADMM_ITERS):
            xp = mpsum.tile([128, 20], F32, tag="mp")
            for g in range(G):
                nc.tensor.matmul(
                    xp[:, g * NW : (g + 1) * NW],
                    lhsT=wt_sb[g],
                    rhs=d1_sb[:, g * NW : (g + 1) * NW],
                    start=True,
                    stop=True,
                )
            msum = state.tile([128, 4], F32, tag="msum")
            nc.vector.reduce_sum(
                msum,
                xp[:, :].rearrange("p (g w) -> p g w", w=NW),
                axis=mybir.AxisListType.X,
            )
            msb = msum[:, :]
            msb_ap = bass.AP(
                tensor=msb.tensor, offset=msb.offset, ap=[msb.ap[0], msb.ap[1], [0, NW]]
            )
            p1 = state.tile([128, 20], F32, tag="p1")
            nc.vector.tensor_tensor(p1, xp, oy_sb, op=AluOpType.add)
            tt_sb = state.tile([128, 20], F32, tag="tt")
            nc.vector.scalar_tensor_tensor(
                out=tt_sb[:, :].rearrange("p (g w) -> p g w", w=NW),
                in0=msb_ap,
                scalar=-1.0 / NW,
                in1=p1[:, :].rearrange("p (g w) -> p g w", w=NW),
                op0=AluOpType.mult,
                op1=AluOpType.add,
            )
            if it == ADMM_ITERS - 1:
                xb_sb = state.tile([128, 20], BF16, tag="xb")
                nc.vector.scalar_tensor_tensor(
                    out=xb_sb[:, :].rearrange("p (g w) -> p g w", w=NW),
                    in0=msb_ap,
                    scalar=-1.0 / NW,
                    in1=xp[:, :].rearrange("p (g w) -> p g w", w=NW),
                    op0=AluOpType.mult,
                    op1=AluOpType.add,
                )
            n2h = state.tile([128, 20], F32, tag="n2h")
            nc.vector.scalar_tensor_tensor(
                out=n2h,
                in0=tt_sb,
                scalar=-1.0,
                in1=h2_sb,
                op0=AluOpType.mult,
                op1=AluOpType.add,
            )
            d1_sb = state.tile([128, 20], F32, tag="d1n")
            nc.vector.tensor_tensor(d1_sb, tt_sb, n2h, op=AluOpType.min)
            if it < ADMM_ITERS - 1:
                pa = state.tile([128, 20], F32, tag="pa")
                nc.vector.tensor_tensor(pa, tt_sb, hmo_sb, op=AluOpType.subtract)
                oy_sb = state.tile([128, 20], F32, tag="oy2")
                nc.vector.tensor_tensor(oy_sb, pa, ohc_sb, op=AluOpType.max)

        # ---- stage 4: compat = S Q^T (after ADMM: overlaps the qt DMA) ----
        # one [128,300] chain per group; rows = group support (25-in-32
        # blocks), cols = group queries (4 x 75). Off-diagonal 25x75 blocks
        # are cross-task garbage, never read downstream.
        compat_sb = []
        with tc.tile_pool(name="cpsum", bufs=2, space="PSUM") as cpsum:
            for g in range(G):
                cp = cpsum.tile([128, GQ], F32, tag="cp")
                for c in range(NCH):
                    nc.tensor.matmul(
                        cp,
                        lhsT=stsl(st_tile, c, g),
                        rhs=qtsl(qt_tile, c, g),
                        start=(c == 0),
                        stop=(c == NCH - 1),
                    )
                cb = cmp_pool.tile([128, GQ], BF16, tag="cb")
                nc.vector.tensor_copy(cb, cp)
                compat_sb.append(cb)

        # ---- stage 5: logits[q, t*5+w] = sum_s compat[s,q] x[s,w] ---------
        out_sb = consts.tile([NQ, T * NW], F32, tag="outsb")
        for t in range(T):
            g, tp = t // GP, t % GP
            rs = slice(tp * 32, tp * 32 + NS)
            qs = slice(tp * NQ, (tp + 1) * NQ)
            lp = lpsum.tile([NQ, NW], F32, tag="lp")
            nc.tensor.matmul(
                lp,
                lhsT=compat_sb[g][rs, qs],
                rhs=xb_sb[rs, g * NW : (g + 1) * NW],
                start=True,
                stop=True,
                tile_position=(tp * 32, 0),
            )
            nc.scalar.activation(
                out_sb[:, t * NW : (t + 1) * NW],
                lp,
                mybir.ActivationFunctionType.Copy,
                scale=scale_sb,
            )
        nc.sync.dma_start(out=out_d[:, :], in_=out_sb)

    _split_waits(nc)
    return nc


_NC_CACHE = None


def _get_nc():
    global _NC_CACHE
    if _NC_CACHE is None:
        _NC_CACHE = _build_program()
    return _NC_CACHE


# ---------------------------------------------------------------------------
def _host_prep(support, query, support_labels, scale):
    """Shard + pack into the DMA layouts. Layout only, no FLOPs."""
    f32 = np.float32
    bf = mybir.dt.np(BF16)
    blk1 = np.zeros((32, 32), dtype=f32)
    blk1[:NS, :NS] = 1.0
    mask = np.kron(np.eye(GP, dtype=f32), blk1)
    blkI = np.zeros((32, 32), dtype=f32)
    blkI[:NS, :NS] = np.eye(NS, dtype=f32)
    beye = np.kron(np.eye(GP, dtype=f32), blkI)
    sc = np.asarray(scale, dtype=f32).reshape(1, 1)

    in_maps = []
    for core in range(N_CORES):
        sl = slice(core * T, (core + 1) * T)
        S = np.asarray(support[sl], dtype=f32)        # [16,25,2560]
        Q = np.asarray(query[sl], dtype=f32)          # [16,75,2560]
        lab = np.asarray(support_labels[sl])          # [16,25] int
        # st col = g*2560 + c*128 + tp*32 + s  (cols 25..31 of each task 0)
        sta = np.zeros((128, G, NCH, GP, 32), dtype=bf)
        sta[..., :NS] = S.reshape(G, GP, NS, NCH, 128).transpose(4, 0, 3, 1, 2)
        st = np.ascontiguousarray(sta.reshape(128, NCH * G * GC))
        # qt col = g*6000 + c*300 + tp*75 + q
        qt = np.ascontiguousarray(
            Q.reshape(G, GP, NQ, NCH, 128).transpose(4, 0, 3, 1, 2)
            .reshape(128, NCH * T * NQ).astype(bf)
        )
        oh = (lab[:, :, None] == np.arange(NW)[None, None, :]).astype(f32)
        # [16,25,5] -> [128,20]: row = tp*32+s, col = g*5+w
        ohp = np.zeros((GP, 32, G, NW), dtype=f32)
        ohp[:, :NS] = oh.reshape(G, GP, NS, NW).transpose(1, 2, 0, 3)
        ohm = ohp.reshape(128, G * NW)
        cf = np.zeros((128, 316), dtype=f32)
        cf[:, C_MASK : C_MASK + 128] = mask
        cf[:, C_EYE : C_EYE + 128] = beye
        cf[:, C_OHC : C_OHC + 20] = ohm / RHO
        cf[:, C_H2 : C_H2 + 20] = 2.0 * (C_REG + 1.0 / RHO) * ohm
        cf[:, C_HMO : C_HMO + 20] = C_REG * ohm
        in_maps.append(
            {
                "st": st,
                "qt": qt,
                "cf": np.ascontiguousarray(cf),
                "scale": sc,
            }
        )
    return in_maps


def _gather(results):
    """Per-core out maps -> full [B_TOT, NQ, NW] logits."""
    outs = []
    for core in range(N_CORES):
        o = np.asarray(results[core]["out"])          # [75, 80]
        outs.append(o.reshape(NQ, T, NW).transpose(1, 0, 2))
    return np.ascontiguousarray(np.concatenate(outs, axis=0), dtype=np.float32)


def kernel(query, support, scale, support_labels, n_way, n_shot):
    assert int(n_way) == NW and int(n_shot) * int(n_way) == NS
    assert query.shape == (B_TOT, NQ, D) and support.shape == (B_TOT, NS, D)
    nc = _get_nc()
    in_maps = _host_prep(support, query, support_labels, scale)
    res = run_bass_kernel_spmd(nc, in_maps, core_ids=list(range(N_CORES)))
    return _gather(res.results)


# revision 23
# speedup vs baseline: 1.1689x; 1.0012x over previous
"""MetaOptNet SVM-CS head on 8 Trainium2 NeuronCores.

Math: the reference runs a 15-iteration Mehrotra interior-point solve of the
Crammer-Singer dual QP per task. Empirically (f64 replication) the IPM is
fully converged by iteration 15, so the target equals the QP optimum. We
compute that optimum with a fixed-matrix ADMM:

    per task:  K = S S^T  (25x25 Gram)
               W~ = rho * (K + (1+rho) I)^{-1}   (Newton-Schulz, 3 iters:
                   2 in bf16 + 1 fp32 polish; |I - cH| <= ~0.1 since
                   9 <= eig(K+9I) <= ~17, and the final fp32 iteration
                   squares the bf16 error away)
               10x ADMM (rho=8), in (d1 = u-y, oy = y+oh/rho) state form:
                   t = center_ways(W~ @ d1) + oy
                   d1' = min(t, 2h - t);  oy' = max(t - (h - oh/rho), oh/rho)
                   where h = (C + 1/rho) oh
               logits = compat^T @ x * scale    (x = center_ways(W~ @ d1),
                   compat = S Q^T computed as one batched PE Gram per group)

The equality constraint A z = 0 (sum over ways per sample) reduces to
centering across ways because A A^T = n_way I; the KKT matrix is way-block-
diagonal with identical blocks K + (1+rho)I, which is what makes the single
25x25 inverse per task sufficient.

Instruction-count shape (the v1 kernel was PE-instruction-overhead bound at
~105us HW: 1072 matmuls of ~200ns each): tasks sit in 32-aligned 25-row
blocks, four tasks per 128-partition tile. Per group the support Gram and
the query/support compat are SINGLE 20-chunk accumulation chains over the
full [128,128] / [128,300] tiles (cross-task blocks are zeroed by a DVE
mask for the Gram, and simply never read for compat), instead of per-task
tile_position matmuls. logits come from compat^T x (contract over the 25
support samples) instead of Q (S^T x) (contract over d=2560), which deletes
the 640 five-column matmuls of v1. Total matmul count ~240 instead of 1072.

Sharding: pure data parallel, 16 tasks per core. Host-side work is layout
only (shard, transpose packing into 128-partition DMA tiles, one-hot
constants); all FLOPs run on-device.

Precision: the QP (Gram, inverse, ADMM) runs in fp32. S/Q ship and contract
in bf16, compat is carried in bf16 — the error enters the output only
linearly (~4e-3 relative on the logits, tolerance 2e-2).

DMA: st/qt are single contiguous [128, n] bf16 tensors (20KB / 48KB per
partition line -> large hardware-DGE descriptors, no small-packet software
descriptor storm); all f32 constants ride in one [128, 316] tensor
(mask | blockdiag-eye | ohc | h2 | hmo); the scalar multiples of blockdiag-I
(2I for Newton-Schulz, 9I for H, 0.065I for X0) are derived on-device.
"""

import sys

sys.path.insert(0, "/opt/trn_rl_repo")

from contextlib import ExitStack

import numpy as np

import concourse.bass as bass
import concourse.tile as tile
from concourse import mybir
from concourse.alu_op_type import AluOpType
from concourse.bass_utils import run_bass_kernel_spmd
from concourse.tile import TileContext

# ---------------------------------------------------------------------------
# Problem constants (hardcoded per the harness contract)
N_CORES = 8
B_TOT = 128
T = 16            # tasks per core
NS = 25           # support samples per task
NW = 5            # ways
NQ = 75           # queries per task
D = 2560          # feature dim
NCH = D // 128    # 20 d-chunks
G = 4             # task groups per core (4 tasks each -> 128-partition tiles)
GP = T // G       # tasks per group
GC = GP * 32      # 128 block columns per group in st (25 used per task)
GQ = GP * NQ      # 300 query columns per group
RHO = 8.0
NS_C = 0.065      # Newton-Schulz init scale for H = K + 9I
NS_ITERS = 3
ADMM_ITERS = 10
C_REG = 0.1

F32 = mybir.dt.float32
BF16 = mybir.dt.bfloat16

# consts_f32 [128, 316] column map
C_MASK = 0        # [128,128] block-diag 25x25 ones at 32-offsets
C_EYE = 128       # [128,128] block-diag I_25 at 32-offsets
C_OHC = 256      # [128,20] oh / rho
C_H2 = 276       # [128,20] 2*(C+1/rho)*oh
C_HMO = 296      # [128,20] C*oh


# ---------------------------------------------------------------------------
# The walrus build here encodes at most ONE sync-wait command per instruction
# (TPB_CTRL / S3_LW setupSyncWait raises "Too many sync wait commands").
# Tile's scheduler freely attaches several waits to one instruction, so after
# scheduling we split the excess onto NoOps inserted immediately before the
# instruction on the same engine — identical semantics, encodable waits.
def _split_waits(nc, max_waits=1):
    cnt = 0
    for blk in nc.m.functions[0].blocks:
        insns = blk.instructions
        idx = 0
        while idx < len(insns):
            ins = insns[idx]
            si = ins.sync_info
            waits = list(si.on_wait) if si and si.on_wait else []
            if len(waits) > max_waits:
                si.on_wait = waits[:max_waits]
                for w in waits[max_waits:]:
                    nop = mybir.InstNoOp(name=f"waitnop_{cnt}", ins=[], outs=[])
                    cnt += 1
                    nop.engine = ins.engine
                    nop.sync_info = mybir.SyncInfo(on_wait=[w], on_update=[])
                    nc.register_instruction(nop, overwrite=True)
                    insns.insert(idx, nop)
                    idx += 1
            idx += 1
    return cnt


# ---------------------------------------------------------------------------
def _build_program(st_chunks=1, qt_chunks=10, n_dma_queues=2):
    nc = bass.Bass("TRN2", target_bir_lowering=False)

    st_d = nc.dram_tensor("st", [128, NCH * G * GC], BF16, kind="ExternalInput")
    qt_d = nc.dram_tensor("qt", [128, NCH * T * NQ], BF16, kind="ExternalInput")
    cf_d = nc.dram_tensor("cf", [128, 316], F32, kind="ExternalInput")
    scale_d = nc.dram_tensor("scale", [1, 1], F32, kind="ExternalInput")
    out_d = nc.dram_tensor("out", [NQ, T * NW], F32, kind="ExternalOutput")

    def stsl(tile_, c, g):
        base = g * (NCH * GC) + c * GC
        return tile_[:, base : base + GC]

    def qtsl(tile_, c, g):
        base = g * (NCH * GQ) + c * GQ
        return tile_[:, base : base + GQ]

    with ExitStack() as ctx:
        tc = ctx.enter_context(TileContext(nc))
        iw_pool = ctx.enter_context(tc.tile_pool(name="iw", bufs=1))
        consts = ctx.enter_context(tc.tile_pool(name="consts", bufs=1))
        mats = ctx.enter_context(tc.tile_pool(name="mats", bufs=12))
        state = ctx.enter_context(tc.tile_pool(name="state", bufs=10))
        cmp_pool = ctx.enter_context(tc.tile_pool(name="cmp", bufs=G))

        # ---- loads --------------------------------------------------------
        cf_sb = consts.tile([128, 316], F32, tag="cf")
        nc.gpsimd.dma_start(out=cf_sb, in_=cf_d[:, :])
        mask_sb = cf_sb[:, C_MASK : C_MASK + 128]
        beye_sb = cf_sb[:, C_EYE : C_EYE + 128]
        ohc_sb = cf_sb[:, C_OHC : C_OHC + 20]
        h2_sb = cf_sb[:, C_H2 : C_H2 + 20]
        hmo_sb = cf_sb[:, C_HMO : C_HMO + 20]

        # chunked DMAs across both trigger queues: moderate per-partition
        # lines (2.5-4KB) keep many partitions streaming concurrently and
        # many independent transfers keep all 16 DMA queues fed (one big
        # transfer measured only ~90-110GB/s; v1-style chunking ~270GB/s).
        # g-major layout so each group's compute can start as its slice lands.
        st_tile = iw_pool.tile([128, NCH * G * GC], BF16, tag="st")
        qt_tile = iw_pool.tile([128, NCH * T * NQ], BF16, tag="qt")
        eng = [nc.sync, nc.gpsimd, nc.scalar, nc.vector][:n_dma_queues]
        ei = 0
        nq = len(eng)
        stw = NCH * GC          # 2560 cols per group (5KB/partition whole)
        for g in range(G):
            for th in range(st_chunks):
                sl_ = slice(g * stw + th * (stw // st_chunks),
                            g * stw + (th + 1) * (stw // st_chunks))
                eng[ei % nq].dma_start(out=st_tile[:, sl_], in_=st_d[:, sl_])
                ei += 1
        qtw = NCH * GQ          # 6000 cols per group
        for g in range(G):
            for th in range(qt_chunks):
                sl_ = slice(g * qtw + th * (qtw // qt_chunks),
                            g * qtw + (th + 1) * (qtw // qt_chunks))
                eng[ei % nq].dma_start(out=qt_tile[:, sl_], in_=qt_d[:, sl_])
                ei += 1

        scale_sb = consts.tile([NQ, 1], F32, tag="scale")
        nc.gpsimd.dma_start(out=scale_sb, in_=scale_d[:, :].to_broadcast([NQ, 1]))

        # bf16 Newton-Schulz init X0 = NS_C * blockdiag-I, derived from beye
        cib_sb = consts.tile([128, 128], BF16, tag="cib")
        nc.scalar.activation(
            cib_sb, beye_sb, mybir.ActivationFunctionType.Copy, scale=NS_C
        )

        # ADMM state: d1 = u - y (init ohc), oy = y + ohc (init ohc)
        d1_sb = state.tile([128, 20], F32, tag="d1")
        nc.vector.tensor_copy(d1_sb, ohc_sb)
        oy_sb = state.tile([128, 20], F32, tag="oy")
        nc.vector.tensor_copy(oy_sb, ohc_sb)

        # ---- stage 1: H_g = mask .* (S S^T) + 9*beye, one chain per group -
        h_all = []
        hb_all = []
        with tc.tile_pool(name="kpsum", bufs=4, space="PSUM") as kpsum:
            for g in range(G):
                kp = kpsum.tile([128, 128], F32, tag="kp")
                for c in range(NCH):
                    nc.tensor.matmul(
                        kp,
                        lhsT=stsl(st_tile, c, g),
                        rhs=stsl(st_tile, c, g),
                        start=(c == 0),
                        stop=(c == NCH - 1),
                    )
                km = mats.tile([128, 128], F32, tag="km")
                nc.vector.tensor_tensor(km, kp, mask_sb, op=AluOpType.mult)
                h_sb = mats.tile([128, 128], F32, tag="h")
                nc.vector.scalar_tensor_tensor(
                    out=h_sb,
                    in0=beye_sb,
                    scalar=1.0 + RHO,
                    in1=km,
                    op0=AluOpType.mult,
                    op1=AluOpType.add,
                )
                h_all.append(h_sb)
                hb = mats.tile([128, 128], BF16, tag="hb")
                nc.vector.tensor_copy(hb, h_sb)
                hb_all.append(hb)

        # ---- stage 2: Newton-Schulz inverse, 4 groups pipelined -----------
        # iters 0-1 in bf16 (NS self-corrects), final iter fp32 squares the
        # bf16 error away (~0.4%^2), so W~ is fp32-quality at 1/4 PE cost.
        wt_sb = []
        with tc.tile_pool(name="npsum", bufs=4, space="PSUM") as npsum:
            x_cur = [cib_sb] * G
            for it in range(NS_ITERS):
                last = it == NS_ITERS - 1
                prev_last = it == NS_ITERS - 2
                for g in range(G):
                    t1p = npsum.tile([128, 128], F32, tag="t1p")
                    nc.tensor.matmul(
                        t1p,
                        lhsT=h_all[g] if last else hb_all[g],
                        rhs=x_cur[g],
                        start=True,
                        stop=True,
                    )
                    u_ns = mats.tile(
                        [128, 128], F32 if last else BF16,
                        tag="u_ns" if last else "u_nsb",
                    )
                    # u = 2*beye - H X
                    nc.vector.scalar_tensor_tensor(
                        out=u_ns,
                        in0=beye_sb,
                        scalar=2.0,
                        in1=t1p,
                        op0=AluOpType.mult,
                        op1=AluOpType.subtract,
                    )
                    x2p = npsum.tile([128, 128], F32, tag="x2p")
                    nc.tensor.matmul(
                        x2p, lhsT=x_cur[g], rhs=u_ns, start=True, stop=True
                    )
                    if last:
                        wt = mats.tile([128, 128], F32, tag="wt")
                        nc.scalar.activation(
                            wt, x2p, mybir.ActivationFunctionType.Copy, scale=RHO
                        )
                        wt_sb.append(wt)
                    else:
                        x_next = mats.tile(
                            [128, 128], F32 if prev_last else BF16,
                            tag="x_ns" if prev_last else "x_nsb",
                        )
                        nc.scalar.activation(
                            x_next, x2p, mybir.ActivationFunctionType.Copy
                        )
                        x_cur[g] = x_next

        # ---- stage 3: ADMM (d1/oy state form) -----------------------------
        # t = center(Wt @ d1) + y + OHC;  d1' = min(t, 2h-t);
        # oy' = max(t - (h - OHC), OHC)
        xb_sb = None
        mpsum = ctx.enter_context(tc.tile_pool(name="mpsum", bufs=2, space="PSUM"))
        lpsum = ctx.enter_context(tc.tile_pool(name="lpsum", bufs=3, space="PSUM"))
        for it in range(ADMM_ITERS):
            xp = mpsum.tile([128, 20], F32, tag="mp")
            for g in range(G):
                nc.tensor.matmul(
                    xp[:, g * NW : (g + 1) * NW],
                    lhsT=wt_sb[g],
                    rhs=d1_sb[:, g * NW : (g + 1) * NW],
                    start=True,
                    stop=True,
                )
            msum = state.tile([128, 4], F32, tag="msum")
            nc.vector.reduce_sum(
                msum,
                xp[:, :].rearrange("p (g w) -> p g w", w=NW),
                axis=mybir.AxisListType.X,
            )
            msb = msum[:, :]
            msb_ap = bass.AP(
                tensor=msb.tensor, offset=msb.offset, ap=[msb.ap[0], msb.ap[1], [0, NW]]
            )
            p1 = state.tile([128, 20], F32, tag="p1")
            nc.vector.tensor_tensor(p1, xp, oy_sb, op=AluOpType.add)
            tt_sb = state.tile([128, 20], F32, tag="tt")
            nc.vector.scalar_tensor_tensor(
                out=tt_sb[:, :].rearrange("p (g w) -> p g w", w=NW),
                in0=msb_ap,
                scalar=-1.0 / NW,
                in1=p1[:, :].rearrange("p (g w) -> p g w", w=NW),
                op0=AluOpType.mult,
                op1=AluOpType.add,
            )
            if it == ADMM_ITERS - 1:
                xb_sb = state.tile([128, 20], BF16, tag="xb")
                nc.vector.scalar_tensor_tensor(
                    out=xb_sb[:, :].rearrange("p (g w) -> p g w", w=NW),
                    in0=msb_ap,
                    scalar=-1.0 / NW,
                    in1=xp[:, :].rearrange("p (g w) -> p g w", w=NW),
                    op0=AluOpType.mult,
                    op1=AluOpType.add,
                )
            n2h = state.tile([128, 20], F32, tag="n2h")
            nc.vector.scalar_tensor_tensor(
                out=n2h,
                in0=tt_sb,
                scalar=-1.0,
                in1=h2_sb,
                op0=AluOpType.mult,
                op1=AluOpType.add,
            )
            d1_sb = state.tile([128, 20], F32, tag="d1n")
            nc.vector.tensor_tensor(d1_sb, tt_sb, n2h, op=AluOpType.min)
            if it < ADMM_ITERS - 1:
                pa = state.tile([128, 20], F32, tag="pa")
                nc.vector.tensor_tensor(pa, tt_sb, hmo_sb, op=AluOpType.subtract)
                oy_sb = state.tile([128, 20], F32, tag="oy2")
                nc.vector.tensor_tensor(oy_sb, pa, ohc_sb, op=AluOpType.max)

        # ---- stage 4: compat = S Q^T (after ADMM: overlaps the qt DMA) ----
        # one [128,300] chain per group; rows = group support (25-in-32
        # blocks), cols = group queries (4 x 75). Off-diagonal 25x75 blocks
        # are cross-task garbage, never read downstream.
        compat_sb = []
        with tc.tile_pool(name="cpsum", bufs=2, space="PSUM") as cpsum:
            for g in range(G):
                cp = cpsum.tile([128, GQ], F32, tag="cp")
                for c in range(NCH):
                    nc.tensor.matmul(
                        cp,
                        lhsT=stsl(st_tile, c, g),
                        rhs=qtsl(qt_tile, c, g),
                        start=(c == 0),
                        stop=(c == NCH - 1),
                    )
                cb = cmp_pool.tile([128, GQ], BF16, tag="cb")
                nc.vector.tensor_copy(cb, cp)
                compat_sb.append(cb)

        # ---- stage 5: logits[q, t*5+w] = sum_s compat[s,q] x[s,w] ---------
        out_sb = consts.tile([NQ, T * NW], F32, tag="outsb")
        for t in range(T):
            g, tp = t // GP, t % GP
            rs = slice(tp * 32, tp * 32 + NS)
            qs = slice(tp * NQ, (tp + 1) * NQ)
            lp = lpsum.tile([NQ, NW], F32, tag="lp")
            nc.tensor.matmul(
                lp,
                lhsT=compat_sb[g][rs, qs],
                rhs=xb_sb[rs, g * NW : (g + 1) * NW],
                start=True,
                stop=True,
                tile_position=(tp * 32, 0),
            )
            nc.scalar.activation(
                out_sb[:, t * NW : (t + 1) * NW],
                lp,
                mybir.ActivationFunctionType.Copy,
                scale=scale_sb,
            )
        nc.sync.dma_start(out=out_d[:, :], in_=out_sb)

    _split_waits(nc)
    return nc


_NC_CACHE = None


def _get_nc():
    global _NC_CACHE
    if _NC_CACHE is None:
        _NC_CACHE = _build_program()
    return _NC_CACHE


# ---------------------------------------------------------------------------
def _host_prep(support, query, support_labels, scale):
    """Shard + pack into the DMA layouts. Layout only, no FLOPs."""
    f32 = np.float32
    bf = mybir.dt.np(BF16)
    blk1 = np.zeros((32, 32), dtype=f32)
    blk1[:NS, :NS] = 1.0
    mask = np.kron(np.eye(GP, dtype=f32), blk1)
    blkI = np.zeros((32, 32), dtype=f32)
    blkI[:NS, :NS] = np.eye(NS, dtype=f32)
    beye = np.kron(np.eye(GP, dtype=f32), blkI)
    sc = np.asarray(scale, dtype=f32).reshape(1, 1)

    in_maps = []
    for core in range(N_CORES):
        sl = slice(core * T, (core + 1) * T)
        S = np.asarray(support[sl], dtype=f32)        # [16,25,2560]
        Q = np.asarray(query[sl], dtype=f32)          # [16,75,2560]
        lab = np.asarray(support_labels[sl])          # [16,25] int
        # st col = g*2560 + c*128 + tp*32 + s  (cols 25..31 of each task 0)
        sta = np.zeros((128, G, NCH, GP, 32), dtype=bf)
        sta[..., :NS] = S.reshape(G, GP, NS, NCH, 128).transpose(4, 0, 3, 1, 2)
        st = np.ascontiguousarray(sta.reshape(128, NCH * G * GC))
        # qt col = g*6000 + c*300 + tp*75 + q
        qt = np.ascontiguousarray(
            Q.reshape(G, GP, NQ, NCH, 128).transpose(4, 0, 3, 1, 2)
            .reshape(128, NCH * T * NQ).astype(bf)
        )
        oh = (lab[:, :, None] == np.arange(NW)[None, None, :]).astype(f32)
        # [16,25,5] -> [128,20]: row = tp*32+s, col = g*5+w
        ohp = np.zeros((GP, 32, G, NW), dtype=f32)
        ohp[:, :NS] = oh.reshape(G, GP, NS, NW).transpose(1, 2, 0, 3)
        ohm = ohp.reshape(128, G * NW)
        cf = np.zeros((128, 316), dtype=f32)
        cf[:, C_MASK : C_MASK + 128] = mask
        cf[:, C_EYE : C_EYE + 128] = beye
        cf[:, C_OHC : C_OHC + 20] = ohm / RHO
        cf[:, C_H2 : C_H2 + 20] = 2.0 * (C_REG + 1.0 / RHO) * ohm
        cf[:, C_HMO : C_HMO + 20] = C_REG * ohm
        in_maps.append(
            {
                "st": st,
                "qt": qt,
                "cf": np.ascontiguousarray(cf),
                "scale": sc,
            }
        )
    return in_maps


def _gather(results):
    """Per-core out maps -> full [B_TOT, NQ, NW] logits."""
    outs = []
    for core in range(N_CORES):
        o = np.asarray(results[core]["out"])          # [75, 80]
        outs.append(o.reshape(NQ, T, NW).transpose(1, 0, 2))
    return np.ascontiguousarray(np.concatenate(outs, axis=0), dtype=np.float32)


def kernel(query, support, scale, support_labels, n_way, n_shot):
    assert int(n_way) == NW and int(n_shot) * int(n_way) == NS
    assert query.shape == (B_TOT, NQ, D) and support.shape == (B_TOT, NS, D)
    nc = _get_nc()
    in_maps = _host_prep(support, query, support_labels, scale)
    res = run_bass_kernel_spmd(nc, in_maps, core_ids=list(range(N_CORES)))
    return _gather(res.results)
